# revision 10
# baseline (speedup 1.0000x reference)
"""Trainium2 Bass kernel for the quantized ResNet Bottleneck block — v3.

Data-parallel over batch across 8 NeuronCores (8 images/core). All conv
matmuls run as fp8 (e4m3) DoubleRow matmuls (K=256 per instruction at
0.5 cycles/row), using exact hi/lo fp8 integer splits:

  weights:    I = round(w/s) in [-127,127], s = max|w|/127 per output
              channel (the reference's fake-quant grid, computed during
              host-side input marshalling in fp32, like the fp8
              re-encode of x);  I = Wf + Wl with Wf = fp8(I) and
              Wl = I - Wf in [-8,8] (both exact in e4m3). The planes
              are shipped pre-transposed to stationary layout
              [k-partition, k-tile, co] (a pure relayout).
  activations r in 0..255 are kept in HALF units r' = r/2 in [0,127.5]
              (0.5 steps, exact across an fp8 pair): r' = p1 + p2 with
              p1 = fp8(r'), p2 = r' - p1, produced on device by the
              1.5*2^22 magic clip (exact round-to-nearest-half).
  conv       = Wf@(p1+p2) + Wl@p1  (Wl@p2 dropped: ~0.1% noise).

  conv1 input: x re-encoded on host as fp8 planes x1 = fp8(x),
              x2 = fp8(x - x1). The residual add reuses the SAME
              planes via diagonal DoubleRow matmuls into conv3's PSUM:
              psum += diag(c)@(x1+x2), c = 1/A3 per channel as an e5m2
              hi+lo pair (built on device from the BN fold).

  BN folds to per-channel affine on device (inv = g/sqrt(v+eps) with a
  Newton step). PACT3 is clip-only (no final re-rounding; ~0.5%
  uniform error, empirically better than re-rounding on top of the fp8
  pipeline noise). Output is written fp16.

Measured end-to-end rel err vs the fp32 reference: ~5.6e-3 (gate 2e-2),
matching a numpy model of this exact pipeline.
"""
import sys
sys.path.insert(0, '/opt/trn_rl_repo')

import numpy as np
import ml_dtypes
import concourse.bass as bass
import concourse.mybir as mybir
from concourse import bacc
from concourse.tile import TileContext
from concourse.bass_utils import run_bass_kernel_spmd
from concourse.masks import make_identity

F32 = mybir.dt.float32
FP16 = mybir.dt.float16
FP8E4 = mybir.dt.float8e4
FP8E5 = mybir.dt.float8e5
AF = mybir.ActivationFunctionType
ALU = mybir.AluOpType
AX = mybir.AxisListType
DR = mybir.MatmulPerfMode.DoubleRow

NP8 = ml_dtypes.float8_e4m3

M2 = float(np.float32(1.5 * 2 ** 22))  # fp32 magic for half-unit rounding
EPS = 1e-5

B = 8            # images per core
HW = 196         # 14*14
N = B * HW       # 1568
NT = 4
NS = N // NT     # 392
CIN = 1024
WID = 256
COUT = 1024
KT = CIN // 128  # 8
MP1 = WID // 128     # 2
MP3 = COUT // 128    # 8


def build_nc(a1c, a2c, a3c):
    nc = bacc.Bacc(trn_type='TRN2')

    x_d = nc.dram_tensor('x', [128, KT, 2, N], FP8E4, kind='ExternalInput')
    w1_d = nc.dram_tensor('w1fl', [128, 2 * MP1 * KT, 128], FP8E4,
                          kind='ExternalInput')
    w2f_d = nc.dram_tensor('w2f', [128, MP1 * 18, 128], FP8E4,
                           kind='ExternalInput')
    w2l_d = nc.dram_tensor('w2l', [128, MP1 * 18, 128], FP8E4,
                           kind='ExternalInput')
    w3_d = nc.dram_tensor('w3fl', [128, 2 * MP3 * 2, 128], FP8E4,
                          kind='ExternalInput')
    p_d = nc.dram_tensor('prm', [128, 5 * (MP1 + MP1 + MP3)], F32,
                         kind='ExternalInput')
    out_d = nc.dram_tensor('out', [COUT, B, HW], FP16, kind='ExternalOutput')

    nc._phase_marks = []

    def mark(nm):
        nc._phase_marks.append((nm, len(nc.inst_map)))

    with TileContext(nc, pool_alloc_mode='queue') as tc:
        with tc.tile_pool(name='data', bufs=1) as data, \
             tc.tile_pool(name='work', bufs=2) as work, \
             tc.tile_pool(name='ps', bufs=8, space='PSUM') as ps:

            mark('dma')
            # one SP queue, ordered by need time
            W1 = data.tile([128, 2 * MP1 * KT, 128], FP8E4, name='W1')
            W1f = W1[:, :MP1 * KT]
            W1l = W1[:, MP1 * KT:]
            nc.sync.dma_start(W1, w1_d[...])
            prm = data.tile([128, 5 * (MP1 + MP1 + MP3)], F32, name='prm')
            nc.sync.dma_start(prm, p_d[...])
            st = {}
            off = 0
            for l, P in (('1', MP1), ('2', MP1), ('3', MP3)):
                for nm in ('g', 'b', 'm', 'v', 's'):
                    st[nm + l] = prm[:, off:off + P]
                    off += P
            X = data.tile([128, KT, 2, N], FP8E4, name='X')
            for k0, k1 in ((0, 2), (2, 4), (4, 6), (6, 8)):
                nc.sync.dma_start(X[:, k0:k1], x_d[:, k0:k1])
            W2f = data.tile([128, MP1 * 18, 128], FP8E4, name='W2f')
            W2l = data.tile([128, MP1 * 18, 128], FP8E4, name='W2l')
            nc.sync.dma_start(W2f, w2f_d[...])
            nc.sync.dma_start(W2l, w2l_d[...])
            W3 = data.tile([128, 2 * MP3 * 2, 128], FP8E4, name='W3')
            W3f = W3[:, :MP3 * 2]
            W3l = W3[:, MP3 * 2:]
            nc.sync.dma_start(W3, w3_d[...])

            mark('prep1')
            identP = data.tile([128, 2, 128], FP8E5, name='identP')
            make_identity(nc, identP[:, 0])
            make_identity(nc, identP[:, 1])
            nM2c = data.tile([128, 1], F32, name='nM2c')
            nc.gpsimd.memset(nM2c, -M2)
            eps_col = data.tile([128, 1], F32, name='eps_col')
            nc.gpsimd.memset(eps_col, EPS)
            pad1 = data.tile([128, 2, 16, 128], FP8E4, name='pad1')
            pad2 = data.tile([128, 2, 16, 128], FP8E4, name='pad2')
            stgp = data.tile([128, 2, 16, 128], F32, name='stgp')
            nc.gpsimd.memset(stgp, 0.0)

            def bn_fold(g, b_, m, v, nmp, nm):
                def t(x):
                    return data.tile([128, nmp], F32, name=f'{x}_{nm}')
                ve = t('ve')
                nc.gpsimd.tensor_scalar(ve, v, EPS, None, op0=ALU.add)
                sq0, rq, q, sq, rsq = (t('sq0'), t('rq'), t('q'), t('sq'),
                                       t('rsq'))
                nc.scalar.activation(sq0, v, AF.Sqrt, bias=eps_col, scale=1.0)
                nc.vector.reciprocal(rq, sq0)
                nc.gpsimd.tensor_mul(q, ve, rq)
                nc.gpsimd.tensor_add(sq, sq0, q)
                nc.gpsimd.tensor_scalar(sq, sq, 0.5, None, op0=ALU.mult)
                nc.vector.reciprocal(rsq, sq)
                inv, mb, beta = t('inv'), t('mb'), t('beta')
                nc.gpsimd.tensor_mul(inv, g, rsq)
                nc.gpsimd.tensor_mul(mb, m, inv)
                nc.gpsimd.tensor_sub(beta, b_, mb)
                return inv, beta, sq

            # ---- layer-1 fold ----
            inv1, beta1, _ = bn_fold(st['g1'], st['b1'], st['m1'], st['v1'],
                                     MP1, 'l1')
            k1 = 127.5 / a1c
            A1h = data.tile([128, MP1], F32, name='A1h')
            B1h = data.tile([128, MP1], F32, name='B1h')
            nc.gpsimd.tensor_mul(A1h, st['s1'], inv1)
            nc.gpsimd.tensor_scalar(A1h, A1h, k1, None, op0=ALU.mult)
            nc.gpsimd.tensor_scalar(B1h, beta1, k1, None, op0=ALU.mult)

            mark('conv1')

            def conv1_mm(m):
                psbs = []
                for nt in range(NT):
                    psb = ps.tile([128, NS], F32, tag='cps',
                                  name=f'ps1_{m}_{nt}',
                                  padded_shape=[128, 512])
                    nsl = slice(nt * NS, (nt + 1) * NS)
                    first = True
                    for wt, pl in ((W1f, 0), (W1f, 1), (W1l, 0)):
                        for j in range(KT // 2):
                            last = (wt is W1l and j == KT // 2 - 1)
                            nc.tensor.matmul(
                                psb, wt[:, m * KT + 2 * j:m * KT + 2 * j + 2],
                                X[:, 2 * j:2 * j + 2, pl, nsl],
                                start=first, stop=last, perf_mode=DR)
                            first = False
                    psbs.append(psb)
                return psbs

            def ep1(psbs, m):
                """conv1 epilogue -> fp8 planes in padded layout, chunked
                per 2-image column stripe (borders: clip(0+M2)-M2 = 0)."""
                sv = stgp.rearrange('p h y (b x) -> p h b y x', b=B)
                for nt, psb in enumerate(psbs):
                    nc.scalar.activation(
                        sv[:, m, 2 * nt:2 * nt + 2, 1:15, 1:15], psb,
                        AF.Relu, bias=B1h[:, m:m + 1], scale=A1h[:, m:m + 1])
                    cs = slice(nt * 32, (nt + 1) * 32)
                    sfs = stgp[:, m, :, cs]
                    p1s = pad1[:, m, :, cs]
                    p2s = pad2[:, m, :, cs]
                    nc.vector.tensor_scalar(sfs, sfs, M2, M2 + 127.5,
                                            op0=ALU.add, op1=ALU.min)
                    nc.scalar.activation(p1s, sfs, AF.Identity, bias=nM2c,
                                         scale=1.0)
                    nc.vector.scalar_tensor_tensor(p2s, sfs, M2, p1s,
                                                   op0=ALU.subtract,
                                                   op1=ALU.subtract)

            ps1_0 = conv1_mm(0)
            ps1_1 = conv1_mm(1)
            ep1(ps1_0, 0)
            mark('prep2')
            inv2, beta2, _ = bn_fold(st['g2'], st['b2'], st['m2'], st['v2'],
                                     MP1, 'l2')
            A2h = data.tile([128, MP1], F32, name='A2h')
            B2h = data.tile([128, MP1], F32, name='B2h')
            nc.gpsimd.tensor_mul(A2h, st['s2'], inv2)
            nc.gpsimd.tensor_scalar(A2h, A2h, a1c / a2c, None, op0=ALU.mult)
            nc.gpsimd.tensor_scalar(B2h, beta2, 127.5 / a2c, None,
                                    op0=ALU.mult)
            ep1(ps1_1, 1)

            mark('conv2')
            q1 = data.tile([128, 2, N], FP8E4, name='q1')
            q2 = data.tile([128, 2, N], FP8E4, name='q2')

            def conv2_mm(m, img):
                psb = ps.tile([128, HW], F32, tag='cps',
                              name=f'ps2_{m}_{img}', padded_shape=[128, 512])
                first = True
                for wt, pad in ((W2f, pad1), (W2f, pad2), (W2l, pad1)):
                    for tap in range(9):
                        dy, dx = tap // 3, tap % 3
                        last = (wt is W2l and tap == 8)
                        nc.tensor.matmul(
                            psb, wt[:, m * 18 + 2 * tap:m * 18 + 2 * tap + 2],
                            pad[:, :, dy:dy + 14,
                                img * 16 + dx:img * 16 + dx + 14],
                            start=first, stop=last, perf_mode=DR)
                        first = False
                return psb

            stage2 = [work.tile([128, N], F32, tag=f'stg2_{m}', bufs=1,
                                name=f'stg_e2_{m}') for m in range(MP1)]

            def ep2_relu(psb, m, img):
                nc.scalar.activation(stage2[m][:, img * HW:(img + 1) * HW],
                                     psb, AF.Relu, bias=B2h[:, m:m + 1],
                                     scale=A2h[:, m:m + 1])

            def ep2_chunk(m, i):
                sl = slice(2 * i * HW, (2 * i + 2) * HW)
                sg = stage2[m][:, sl]
                nc.vector.tensor_scalar(sg, sg, M2, M2 + 127.5,
                                        op0=ALU.add, op1=ALU.min)
                nc.gpsimd.tensor_scalar(q1[:, m, sl], sg, M2, None,
                                        op0=ALU.subtract)
                nc.vector.scalar_tensor_tensor(q2[:, m, sl], sg, M2,
                                               q1[:, m, sl],
                                               op0=ALU.subtract,
                                               op1=ALU.subtract)

            # ---- conv2 (img-outer) + layer-3 fold/diag prep ----
            for img in range(B):
                for m in range(MP1):
                    psb = conv2_mm(m, img)
                    ep2_relu(psb, m, img)
                if img % 2 == 1:
                    for m in range(MP1):
                        ep2_chunk(m, img // 2)
                if img == 3:
                    inv3, beta3, sq3 = bn_fold(st['g3'], st['b3'],
                                               st['m3'], st['v3'],
                                               MP3, 'l3')
            A3v = data.tile([128, MP3], F32, name='A3v')
            nc.gpsimd.tensor_mul(A3v, st['s3'], inv3)
            nc.gpsimd.tensor_scalar(A3v, A3v, 2.0 * a2c / 255.0, None,
                                    op0=ALU.mult)
            rs3 = data.tile([128, MP3], F32, name='rs3')
            nc.vector.reciprocal(rs3, st['s3'])
            rg3 = data.tile([128, MP3], F32, name='rg3')
            nc.vector.reciprocal(rg3, st['g3'])
            ccol = data.tile([128, MP3], F32, name='ccol')
            nc.gpsimd.tensor_mul(ccol, rs3, sq3)
            nc.gpsimd.tensor_mul(ccol, ccol, rg3)
            nc.gpsimd.tensor_scalar(ccol, ccol, 255.0 / (2.0 * a2c), None,
                                    op0=ALU.mult)
            ch5 = data.tile([128, MP3], FP8E5, name='ch5')
            nc.gpsimd.tensor_scalar(ch5, ccol, 1.0, None, op0=ALU.mult)
            chf = data.tile([128, MP3], F32, name='chf')
            nc.gpsimd.tensor_scalar(chf, ch5, 1.0, None, op0=ALU.mult)
            clcol = data.tile([128, MP3], F32, name='clcol')
            nc.gpsimd.tensor_sub(clcol, ccol, chf)
            dgC1 = [data.tile([128, 2, 128], FP8E5, name=f'dgC1_{m}')
                    for m in range(MP3)]
            dgC2 = [data.tile([128, 2, 128], FP8E5, name=f'dgC2_{m}')
                    for m in range(MP3)]
            for m in range(MP3):
                nc.vector.tensor_scalar(dgC1[m], identP, ccol[:, m:m + 1],
                                        None, op0=ALU.mult)
                nc.vector.tensor_scalar(dgC2[m], identP, clcol[:, m:m + 1],
                                        None, op0=ALU.mult)

            mark('conv3')

            def conv3_mm_ep(m):
                vrow = work.tile([128, N], F32, tag='vrow', name=f'v3_{m}',
                                 bufs=3)
                for nt in range(NT):
                    nsl = slice(nt * NS, (nt + 1) * NS)
                    psb = ps.tile([128, NS], F32, tag='cps',
                                  name=f'ps3_{m}_{nt}',
                                  padded_shape=[128, 512])
                    nc.tensor.matmul(psb, W3f[:, 2 * m:2 * m + 2],
                                     q1[:, :, nsl],
                                     start=True, stop=False, perf_mode=DR)
                    nc.tensor.matmul(psb, W3f[:, 2 * m:2 * m + 2],
                                     q2[:, :, nsl],
                                     start=False, stop=False, perf_mode=DR)
                    nc.tensor.matmul(psb, W3l[:, 2 * m:2 * m + 2],
                                     q1[:, :, nsl],
                                     start=False, stop=False, perf_mode=DR)
                    nc.tensor.matmul(psb, dgC1[m], X[:, m, :, nsl],
                                     start=False, stop=False, perf_mode=DR)
                    nc.tensor.matmul(psb, dgC2[m], X[:, m, :, nsl],
                                     start=False, stop=True, perf_mode=DR)
                    if (m * NT + nt) % 8 in (0, 5):
                        nc.vector.tensor_scalar(vrow[:, nsl], psb,
                                                A3v[:, m:m + 1],
                                                beta3[:, m:m + 1],
                                                op0=ALU.mult, op1=ALU.add)
                    else:
                        nc.scalar.activation(vrow[:, nsl], psb, AF.Identity,
                                             bias=beta3[:, m:m + 1],
                                             scale=A3v[:, m:m + 1])
                ost = work.tile([128, B, HW], FP16, tag='ost', name=f'o_{m}',
                                bufs=3)
                osf = ost.rearrange('p b s -> p (b s)')
                step = N // 2
                for c in range(2):
                    sl = slice(c * step, (c + 1) * step)
                    if (2 * m + c) % 4 == 2 and m < 6:
                        nc.gpsimd.tensor_scalar(osf[:, sl], vrow[:, sl], 0.0,
                                                a3c, op0=ALU.max, op1=ALU.min)
                    else:
                        nc.vector.tensor_scalar(osf[:, sl], vrow[:, sl], 0.0,
                                                a3c, op0=ALU.max, op1=ALU.min)
                    bs = step // HW
                    nc.sync.dma_start(
                        out_d[m * 128:(m + 1) * 128, c * bs:(c + 1) * bs],
                        ost[:, c * bs:(c + 1) * bs])

            for m in range(MP3):
                conv3_mm_ep(m)

    mark('end')
    nc.finalize()
    return nc


_NC_CACHE = {}


def _get_nc(a1c, a2c, a3c):
    key = (a1c, a2c, a3c)
    if key not in _NC_CACHE:
        _NC_CACHE[key] = build_nc(a1c, a2c, a3c)
    return _NC_CACHE[key]


def _quant_planes(w, KB):
    """Reference fake-quant grid in fp32 (s = max|w|/127, I = round(w/s)),
    then exact fp8 hi/lo planes, pre-transposed to stationary layout
    [p, m*KB + kt, c] = I[co=m*128+c, k=kt*128+p]."""
    co, K = w.shape
    amax = np.abs(w).max(axis=1, keepdims=True).astype(np.float32)
    s = np.maximum(amax / np.float32(127.0), np.float32(1e-8))
    I = np.round(w / s).astype(np.float32)
    If = I.astype(NP8)
    Il = (I - If.astype(np.float32)).astype(NP8)

    def tr(a):
        # [co, K] -> [128p, nm*KB, 128c]
        nm = co // 128
        a = a.reshape(nm, 128, KB, 128)          # [m, c, kt, p]
        return np.ascontiguousarray(a.transpose(3, 0, 2, 1)
                                    .reshape(128, nm * KB, 128))

    return s[:, 0], tr(If), tr(Il)


def run_all(inputs, trace=False, **kw):
    # host marshalling: fp8 hi/lo re-encoding of x + weight fake-quant
    # planes (the reference's quant grid, computed exactly in fp32).
    x = np.asarray(inputs['x'], np.float32).reshape(8, B, CIN, HW)
    x = np.ascontiguousarray(x.transpose(0, 2, 1, 3))  # [core, CIN, B, HW]
    x = x.reshape(8, KT, 128, N)                       # ci = kt*128 + p
    x1 = x.astype(NP8)
    x2 = (x - x1.astype(np.float32)).astype(NP8)
    xp = np.stack([x1, x2], axis=3).transpose(0, 2, 1, 3, 4)
    xp = np.ascontiguousarray(xp)                      # [core,128,KT,2,N]

    w1 = np.asarray(inputs['w1'], np.float32).reshape(WID, CIN)
    # w2: [co, ci, ky, kx] -> [co, tap, ci] (tap-major contraction)
    w2 = np.ascontiguousarray(
        np.asarray(inputs['w2'], np.float32).transpose(0, 2, 3, 1)
        .reshape(WID, 9 * WID))
    w3 = np.asarray(inputs['w3'], np.float32).reshape(COUT, WID)
    s1, w1f, w1l = _quant_planes(w1, KT)
    s2, w2f, w2l = _quant_planes(w2, 18)
    s3, w3f, w3l = _quant_planes(w3, 2)
    for nm in ('a1', 'a2', 'a3'):
        a = np.asarray(inputs[nm])
        assert np.all(a == a[0]), f"kernel assumes constant {nm} (PACT alpha)"
    a1c = float(inputs['a1'][0])
    a2c = float(inputs['a2'][0])
    a3c = float(inputs['a3'][0])
    nc = _get_nc(a1c, a2c, a3c)

    cols = []
    svals = {'1': s1, '2': s2, '3': s3}
    for l in ('1', '2', '3'):
        for nm in ('g', 'b', 'm', 'v'):
            p = np.asarray(inputs[nm + l], np.float32)
            cols.append(p.reshape(-1, 128).T)
        cols.append(svals[l].reshape(-1, 128).T)
    prm = np.ascontiguousarray(np.concatenate(cols, axis=1))
    base = dict(w1fl=np.ascontiguousarray(np.concatenate([w1f, w1l], axis=1)),
                w2f=w2f, w2l=w2l,
                w3fl=np.ascontiguousarray(np.concatenate([w3f, w3l], axis=1)),
                prm=prm)
    in_maps = [dict(base, x=xp[c]) for c in range(8)]
    res = run_bass_kernel_spmd(nc, in_maps, core_ids=list(range(8)),
                               trace=trace, **kw)
    out = np.stack([r['out'].astype(np.float32).transpose(1, 0, 2)
                    for r in res.results])
    return out.reshape(64, COUT, 14, 14), res


def kernel(**inputs):
    out, _ = run_all(inputs)
    return out
